# revision 18
# baseline (speedup 1.0000x reference)
"""Trainium2 Bass kernel for nn_DSAGPredictor (dense transposed-softmax attention).

Math (b=1, C=256, H=W=96, n=9216, Z=16):
  xf = x.reshape(256, n)
  q = Wq@xf ; k = Wk@xf ; v = Wv@xf
  S = k^T q                      [n_k, n_q]
  A = softmax(S, axis=q)         (row-normalized over the q axis)
  Y = v @ A
  rel = embd[isWithin, dist+8]   [16, 256]
  pos = rel @ xf                 [16, n]
  final[z] = Wproj[:, :256] @ (Y + x) + Wproj[:, 256] (outer) pos[z]

Algebra used (v2):
  - G = Wk^T Wq folded on host: S = x^T (G xq). The k projection disappears;
    x tiles are the S-matmul stationary operand directly.
  - WprojC folded into v: v2 = WprojC @ Wv, so base = (v2 x) A + WprojC xq
    accumulates into one SBUF accumulator.
  - Fixed-shift softmax: P = exp(S - C_SHIFT), s_k = global rowsum over q via
    SIX small AllReduces (one per 12 k-tiles), A = diag(1/s) P with 1/s folded
    into v2^T columns. Rowsums computed on the vector engine (tensor_reduce)
    to keep the scalar engine free for the exps.

Sharding: q (token) axis split across 8 cores (1152 columns each). Output is
column-sharded; the host concatenates.

Schedule: single fused loop over 72 k-tiles. Per tile: S matmuls (f32r),
exp on scalar engine, rowsum on vector, spill P (bf16) to DRAM (last 8
k-tiles stay in SBUF). v2^T projection tiles + the WprojC xq base term are
interleaved under the first third; P.V segments (PSUM accumulate + SBUF
dump-add) are interleaved as their AllReduce segment completes. Tail: last
two PV segments, then the z rank-1 updates with 4-z-batched drains and
output DMA split across engine queues.
"""
import os

from contextlib import ExitStack

import ml_dtypes
import numpy as np

import concourse.bass as bass
import concourse.bacc as bacc
import concourse.tile as tile
from concourse import mybir, bass_utils
from concourse.bass import broadcast_tensor_aps

N_CORES = 8
CDIM = 256          # channels
N_TOK = 9216        # hh*ww
NQ = N_TOK // N_CORES   # per-core q slice = 1152
QCH = 384           # q chunk (>=256 keeps f32r at full PE rate)
NCH = NQ // QCH     # 3 chunks per core
NKT = N_TOK // 128  # 72 k-tiles
ZDIM = 16
MAXL = 8
C_SHIFT = 96.0

SEG = 12
AR_SEGS = [(i * SEG, (i + 1) * SEG) for i in range(NKT // SEG)]  # 6 segments
PK0 = 66            # kt >= PK0: P stays in SBUF (no spill)

f32 = mybir.dt.float32
f32r = mybir.dt.float32r
bf16 = mybir.dt.bfloat16

_CACHE = {}


DEBUG_DUMP = bool(int(os.environ.get("KERNEL_DEBUG_DUMP", "0")))


def _build_nc():
    nc = bacc.Bacc("TRN2", target_bir_lowering=False, debug=False,
                   num_devices=N_CORES)

    # ---- I/O (f32r tensors receive plain f32 bits; PE rounds internally) ----
    xf_d = nc.dram_tensor("xf", [2, 128, N_TOK], f32r, kind="ExternalInput")
    xq_d = nc.dram_tensor("xq", [2, 128, NQ], f32r, kind="ExternalInput")
    gT_d = nc.dram_tensor("gT", [2, 128, CDIM], f32r, kind="ExternalInput")
    wv2T_d = nc.dram_tensor("wv2T", [2, 128, CDIM], f32r, kind="ExternalInput")
    wpT_d = nc.dram_tensor("wpT", [2, 128, CDIM], f32r, kind="ExternalInput")
    relT_d = nc.dram_tensor("relT", [2, 128, ZDIM], f32r, kind="ExternalInput")
    wlast_d = nc.dram_tensor("wlast", [1, CDIM], bf16, kind="ExternalInput")
    out_d = nc.dram_tensor("out", [ZDIM, 2, 128, NQ], f32, kind="ExternalOutput")
    if DEBUG_DUMP:
        dbg_y = nc.dram_tensor("dbg_y", [128, 2, NQ], f32, kind="ExternalOutput")
        dbg_st = nc.dram_tensor("dbg_st", [128, NKT], f32, kind="ExternalOutput")
        dbg_v2 = nc.dram_tensor("dbg_v2", [128, NKT, CDIM], bf16,
                                kind="ExternalOutput")
        dbg_qg = nc.dram_tensor("dbg_qg", [128, 2, NQ], f32, kind="ExternalOutput")

    with tile.TileContext(nc) as tc, ExitStack() as ctx:
        # ---- pools (SBUF) ----
        const = ctx.enter_context(tc.tile_pool(name="const", bufs=1))
        big = ctx.enter_context(tc.tile_pool(name="big", bufs=1))
        pout = ctx.enter_context(tc.tile_pool(name="pout", bufs=3))
        pin = ctx.enter_context(tc.tile_pool(name="pin", bufs=2))
        posp = ctx.enter_context(tc.tile_pool(name="posp", bufs=1))
        opool = ctx.enter_context(tc.tile_pool(name="opool", bufs=4))
        dram = ctx.enter_context(tc.tile_pool(name="dram", bufs=1, space="DRAM"))

        # ---- persistent SBUF ----
        gT_r = const.tile([128, 2, CDIM], f32r)
        wv2_r = const.tile([128, 2, CDIM], f32r)
        wp_r = const.tile([128, 2, CDIM], f32r)
        rel_r = const.tile([128, 2, ZDIM], f32r)
        wl_b = const.tile([1, CDIM], bf16)
        negc = const.tile([128, 1], f32)

        xq_r = big.tile([128, 2, NQ], f32r)
        qg_s = big.tile([128, 2, NQ], f32r)           # G @ xq
        xch = [big.tile([128, 2, 512], f32r, name=f"xch{i}")
               for i in range(N_TOK // 512)]          # x, channel-major chunks
        v2t_s = big.tile([128, NKT, CDIM], bf16)      # v2^T, token-major
        y_acc = big.tile([128, 2, NQ], f32)           # base accumulator
        pk = big.tile([128, NKT - PK0, NQ], bf16)     # P kept in SBUF
        stats = big.tile([128, NKT], f32)             # local rowsums per ktile
        stats_tot = big.tile([128, NKT], f32)
        recip = big.tile([128, NKT], f32)

        # ---- DRAM scratch ----
        pspill = [dram.tile([min(hi, PK0) - lo, 128, NQ], bf16,
                            name=f"pspill{i}")
                  for i, (lo, hi) in enumerate(AR_SEGS) if lo < PK0]
        pos_d = dram.tile([ZDIM, NQ], bf16)
        cc_in = [dram.tile([128, hi - lo], f32, name=f"cc_in{i}")
                 for i, (lo, hi) in enumerate(AR_SEGS)]
        cc_out = [dram.tile([128, hi - lo], f32, addr_space="Shared",
                            name=f"cc_out{i}")
                  for i, (lo, hi) in enumerate(AR_SEGS)]

        # ---- input DMAs (sync queue) ----
        nc.sync.dma_start(xq_r[:], xq_d[:, :, :].rearrange("h p c -> p h c"))
        nc.sync.dma_start(gT_r[:], gT_d[:, :, :].rearrange("h p c -> p h c"))
        nc.sync.dma_start(rel_r[:], relT_d[:, :, :].rearrange("h p c -> p h c"))
        nc.sync.dma_start(wv2_r[:], wv2T_d[:, :, :].rearrange("h p c -> p h c"))
        nc.sync.dma_start(wp_r[:], wpT_d[:, :, :].rearrange("h p c -> p h c"))
        nc.sync.dma_start(wl_b[:], wlast_d[:, :])
        for i in range(N_TOK // 512):
            sl = slice(i * 512, (i + 1) * 512)
            nc.sync.dma_start(xch[i][:],
                              xf_d[:, :, sl].rearrange("h p c -> p h c"))
        nc.vector.memset(negc[:], -C_SHIFT)

        # ---- helpers ----
        def _ar_seg(idx):
            """AllReduce one stats segment; reciprocal + fold into v2T."""
            lo, hi = AR_SEGS[idx]
            hs = slice(lo, hi)
            nc.gpsimd.dma_start(cc_in[idx][:], stats[:, hs])
            nc.gpsimd.collective_compute(
                "AllReduce",
                mybir.AluOpType.add,
                replica_groups=[list(range(N_CORES))],
                ins=[cc_in[idx][:].opt()],
                outs=[cc_out[idx][:].opt()],
            )
            nc.gpsimd.dma_start(stats_tot[:, hs], cc_out[idx][:])
            nc.vector.reciprocal(recip[:, hs], stats_tot[:, hs])
            for kt in range(lo, hi):
                nc.gpsimd.tensor_scalar_mul(v2t_s[:, kt, :], v2t_s[:, kt, :],
                                            recip[:, kt:kt + 1])

        def _s_unit(kt, psS):
            """S matmuls + exp + rowsum (+ spill) for one k-tile."""
            tch, ms = divmod(kt, 4)
            msl = slice(ms * 128, (ms + 1) * 128)
            xt = xch[tch]
            ps_s = psS.tile([128, NCH, 512], f32, tag="s")
            for qc in range(NCH):
                qsl = slice(qc * QCH, (qc + 1) * QCH)
                nc.tensor.matmul(ps_s[:, qc, 0:QCH], xt[:, 0, msl],
                                 qg_s[:, 0, qsl], start=True, stop=False)
                nc.tensor.matmul(ps_s[:, qc, 0:QCH], xt[:, 1, msl],
                                 qg_s[:, 1, qsl], start=False, stop=True)
            if kt < PK0:
                pt = pout.tile([128, NQ], bf16, tag="pt")
                pt3 = pt[:].rearrange("p (c q) -> p c q", c=NCH)
                pt2 = pt[:]
            else:
                pt2 = pk[:, kt - PK0, :]
                pt3 = pt2.rearrange("p (c q) -> p c q", c=NCH)
            nc.scalar.activation(pt3, ps_s[:, :, 0:QCH],
                                 mybir.ActivationFunctionType.Exp,
                                 bias=negc[:], scale=1.0,
                                 accum_out=stats[:, kt:kt + 1])
            if kt < PK0:
                seg = kt // SEG
                nc.sync.dma_start(pspill[seg][kt - AR_SEGS[seg][0]], pt2)

        def _v2t_unit(kt, pool):
            tch, ms = divmod(kt, 4)
            msl = slice(ms * 128, (ms + 1) * 128)
            ps_v = pool.tile([128, 512], f32, tag="aux")
            for h in range(2):
                nc.tensor.matmul(ps_v[:, 0:CDIM], xch[tch][:, h, msl],
                                 wv2_r[:, h, :], start=(h == 0), stop=(h == 1))
            nc.vector.tensor_copy(v2t_s[:, kt, :], ps_v[:, 0:CDIM])

        def _wp_unit(qc, pool):
            """WprojC @ xq base term -> initializes y_acc[:, :, qsl]."""
            qsl = slice(qc * QCH, (qc + 1) * QCH)
            for oh in range(2):
                ohs = slice(oh * 128, (oh + 1) * 128)
                ps = pool.tile([128, 512], f32, tag="aux")
                for h in range(2):
                    nc.tensor.matmul(ps[:, 0:QCH], wp_r[:, h, ohs],
                                     xq_r[:, h, qsl], start=(h == 0),
                                     stop=(h == 1))
                nc.vector.tensor_copy(y_acc[:, oh, qsl], ps[:, 0:QCH])

        def _pv_load(s, qc):
            """Prefetch spilled P rows for PV unit (seg s, chunk qc)."""
            lo, hi = AR_SEGS[s]
            hi = min(hi, PK0)
            if hi <= lo:
                return None
            qsl = slice(qc * QCH, (qc + 1) * QCH)
            ptg = pin.tile([128, hi - lo, QCH], bf16, tag="ptg",
                           name=f"ptg{s}_{qc}")
            nc.sync.dma_start(
                ptg[:], pspill[s][:, :, qsl].rearrange("g p c -> p g c"))
            return ptg

        def _pv_unit(s, qc, ptg, pool):
            """PV psum-accumulate over seg s for q-chunk qc; dump-add y_acc."""
            lo, hi = AR_SEGS[s]
            qsl = slice(qc * QCH, (qc + 1) * QCH)
            ps = pool.tile([128, 2, 512], f32, tag="pv")
            for j, kt in enumerate(range(lo, hi)):
                if kt < PK0:
                    rhs = ptg[:, j, :]
                else:
                    rhs = pk[:, kt - PK0, qsl]
                for oh in range(2):
                    ohs = slice(oh * 128, (oh + 1) * 128)
                    nc.tensor.matmul(ps[:, oh, 0:QCH], v2t_s[:, kt, ohs],
                                     rhs, start=(kt == lo), stop=(kt == hi - 1))
            nc.vector.tensor_add(y_acc[:, :, qsl], ps[:, :, 0:QCH],
                                 y_acc[:, :, qsl])

        # PV interleave schedule: seg s q-chunk qc emitted after S k-tile
        pv_at = {}
        for s in range(4):
            for qc in range(NCH):
                pv_at[min(30 + SEG * s + 4 * qc, 63 + 2 * s + qc)] = (s, qc)

        # ---- main fused loop ----
        pv_tiles = {}
        with tc.tile_pool(name="psS", bufs=2, space="PSUM") as psS:
            with tc.tile_pool(name="psAux", bufs=2, space="PSUM") as psAux:
                # qg = G @ xq  (channel-major, f32r)
                for qc in range(NCH):
                    qsl = slice(qc * QCH, (qc + 1) * QCH)
                    for h in range(2):
                        hs = slice(h * 128, (h + 1) * 128)
                        ps_q = psAux.tile([128, 512], f32, tag="aux")
                        nc.tensor.matmul(ps_q[:, 0:QCH], gT_r[:, 0, hs],
                                         xq_r[:, 0, qsl], start=True, stop=False)
                        nc.tensor.matmul(ps_q[:, 0:QCH], gT_r[:, 1, hs],
                                         xq_r[:, 1, qsl], start=False, stop=True)
                        nc.vector.tensor_copy(qg_s[:, h, qsl], ps_q[:, 0:QCH])
                    # pos chunk
                    ps_p = psAux.tile([ZDIM, 512], f32, tag="aux")
                    nc.tensor.matmul(ps_p[:, 0:QCH], rel_r[:, 0, :],
                                     xq_r[:, 0, qsl], start=True, stop=False)
                    nc.tensor.matmul(ps_p[:, 0:QCH], rel_r[:, 1, :],
                                     xq_r[:, 1, qsl], start=False, stop=True)
                    pos16 = pout.tile([ZDIM, QCH], bf16, tag="pos16")
                    nc.vector.tensor_copy(pos16[:], ps_p[:, 0:QCH])
                    nc.sync.dma_start(pos_d[:, qsl], pos16[:])
                # k-tiles 0..23: S + v2t projection + wp base
                for kt in range(24):
                    _s_unit(kt, psS)
                    if kt < 18:
                        for kv in range(4 * kt, 4 * kt + 4):
                            _v2t_unit(kv, psAux)
                    if kt in (18, 20, 22):
                        _wp_unit((kt - 18) // 2, psAux)
                    if kt % SEG == SEG - 1:
                        _ar_seg(kt // SEG)
            with tc.tile_pool(name="psPV", bufs=1, space="PSUM") as psPV:
                # k-tiles 24..71: S + interleaved PV segments 0..3
                for kt in range(24, NKT):
                    if kt + 2 in pv_at:
                        s, qc = pv_at[kt + 2]
                        pv_tiles[(s, qc)] = _pv_load(s, qc)
                    _s_unit(kt, psS)
                    if kt in pv_at:
                        s, qc = pv_at[kt]
                        _pv_unit(s, qc, pv_tiles.pop((s, qc)), psPV)
                    if kt % SEG == SEG - 1:
                        _ar_seg(kt // SEG)

        # ---- tail: PV segments 4..5 + z rank-1 + output, interleaved ----
        def _z_units(qc, psZ):
            qsl = slice(qc * QCH, (qc + 1) * QCH)
            posc = posp.tile([1, ZDIM * QCH], bf16, tag="posc",
                             name=f"posc{qc}")
            nc.sync.dma_start(
                posc[:].rearrange("p (z t) -> p z t", z=ZDIM)[0:1, :, :],
                pos_d[:, qsl])
            oq = [nc.sync, nc.scalar, nc.gpsimd]
            for z in range(ZDIM):
                poz = slice(z * QCH, (z + 1) * QCH)
                for oh in range(2):
                    u = z * 2 + oh
                    ps_o = psZ.tile([128, 512], f32, tag="z")
                    nc.tensor.matmul(
                        ps_o[:, 0:QCH], wl_b[0:1, oh * 128:(oh + 1) * 128],
                        posc[0:1, poz], start=True, stop=True)
                    ot = opool.tile([128, QCH], f32, tag="ot")
                    if u % 2 == 0:
                        nc.vector.tensor_add(ot[:], ps_o[:, 0:QCH],
                                             y_acc[:, oh, qsl])
                    else:
                        tmp = opool.tile([128, QCH], f32, tag="tmp")
                        nc.scalar.copy(tmp[:], ps_o[:, 0:QCH])
                        nc.gpsimd.tensor_add(ot[:], tmp[:], y_acc[:, oh, qsl])
                    oq[u % 3].dma_start(out_d[z, oh, :, qsl], ot[:])

        with tc.tile_pool(name="psT", bufs=2, space="PSUM") as psT:
            with tc.tile_pool(name="psZ", bufs=4, space="PSUM") as psZ:
                for qc in range(NCH):
                    pv_tiles[(4, qc)] = _pv_load(4, qc)
                pv_tiles[(5, 0)] = _pv_load(5, 0)
                for qc in range(NCH):
                    _pv_unit(4, qc, pv_tiles.pop((4, qc)), psT)
                for qc in range(NCH):
                    if qc + 1 < NCH:
                        pv_tiles[(5, qc + 1)] = _pv_load(5, qc + 1)
                    _pv_unit(5, qc, pv_tiles.pop((5, qc)), psT)
                    _z_units(qc, psZ)

        if DEBUG_DUMP:
            nc.sync.dma_start(dbg_y[:, :, :], y_acc[:])
            nc.sync.dma_start(dbg_st[:, :], stats_tot[:])
            nc.sync.dma_start(dbg_v2[:, :, :], v2t_s[:])
            nc.sync.dma_start(dbg_qg[:, :, :], qg_s[:].bitcast(f32))

    nc.compile()
    return nc


def _get_nc():
    if "nc" not in _CACHE:
        _CACHE["nc"] = _build_nc()
    return _CACHE["nc"]


def _prep_in_maps(x, Wq, Wk, Wv, embd, Wproj, dist, isWithin):
    x = np.asarray(x, np.float32)
    Wq = np.asarray(Wq, np.float32)
    Wk = np.asarray(Wk, np.float32)
    Wv = np.asarray(Wv, np.float32)
    embd = np.asarray(embd, np.float32)
    Wproj = np.asarray(Wproj, np.float32)
    dist = np.asarray(dist).astype(np.int64)
    isWithin = np.asarray(isWithin).astype(np.int64)

    xf = np.ascontiguousarray(x.reshape(CDIM, N_TOK))
    WprojC = Wproj[:, :CDIM]
    wlast = np.ascontiguousarray(Wproj[:, CDIM]).reshape(1, CDIM)
    Wv2 = WprojC @ Wv
    rel = embd[isWithin, dist + MAXL]            # [16, 256]
    gT = Wq.T @ Wk                               # (Wk^T Wq)^T

    def split2(a):  # [256, m] -> [2, 128, m]
        return np.ascontiguousarray(a.reshape(2, 128, -1), dtype=np.float32)

    common = {
        "xf": split2(xf),
        "gT": split2(gT),
        "wv2T": split2(Wv2.T),
        "wpT": split2(WprojC.T),
        "relT": split2(rel.T),
        "wlast": wlast.astype(ml_dtypes.bfloat16),
    }
    in_maps = []
    for c in range(N_CORES):
        m = dict(common)
        m["xq"] = split2(np.ascontiguousarray(xf[:, c * NQ:(c + 1) * NQ]))
        in_maps.append(m)
    return in_maps


def run(inputs, trace=False, tmpdir=None):
    nc = _get_nc()
    in_maps = _prep_in_maps(**inputs)
    res = bass_utils.run_bass_kernel_spmd(
        nc, in_maps, core_ids=list(range(N_CORES)), trace=trace, tmpdir=tmpdir,
    )
    parts = [res.results[c]["out"].reshape(ZDIM, CDIM, NQ)
             for c in range(N_CORES)]
    full = np.concatenate(parts, axis=2).reshape(ZDIM, CDIM, 96, 96)
    return np.ascontiguousarray(full.astype(np.float32)), res


def kernel(**inputs) -> np.ndarray:
    out, _ = run(inputs, trace=bool(int(os.environ.get("KERNEL_TRACE", "0"))))
    return out
